# revision 10
# baseline (speedup 1.0000x reference)
"""CoAttention Trainium2 Bass kernel (single-exp, all-bf16, 3 DMAs).

Sharding: data-parallel over batch B=8 across the 8 NeuronCores (one batch
element per core); CxC projection weights replicated.

Per-core math (x1, x2 are [C, L] channel-major slices):
  qT = Wq x1 + bq  [C,L];  kT = Wk x2 + bk  [C,L]
  v1 = x1^T Wv1^T  [L,C];  v2 = x2^T Wv2^T  [L,C]   (v-biases fold into xT)
  E  = exp((qT^T kT)/sqrt(C))   [q,k] tiles, bf16, SBUF-resident (8MB)
  ETc = per-q-chunk PE transposes of E  [k,q]
  vk[q,c] = (ETc^T@v2)/d_row;  out1 = LN(vk + x1^T + b_v2)^T
  vq[k,c] = (E^T@v1)/d_col;    out2 = LN(vq + x2^T + b_v1)^T

Key choices:
- THREE DMA transfers total: one packed, partition-major bf16 load (x^T
  residual tiles, identity, ones, all biases, pre-transposed weights) and one
  bf16 store per output (outputs staged fully in SBUF; the host reorders and
  upcasts to f32). Channel-major x is rebuilt on-chip with PE transposes.
  DMA count dominates the wall clock in this environment.
- One exp pass serves both softmaxes: the row softmax uses E directly as the
  stationary matmul operand, the column softmax uses PE-transposed chunk
  tiles; softmax denominators ride free on activation accum_out.
- All biases ride in the packed load: q/k biases enter via a rank-1
  (bias x ones) matmul accumulated onto the projection PSUM chain; v biases
  are broadcast on-chip by one rank-1 matmul each.
- Everything on-chip is bf16 (tolerance is 2e-2 rel; this keeps ~3e-3).
  Softmax max-subtraction is skipped: logits ~ N(0,1), exp is safe in f32.
- LN((psum)/d + xT) is one fused DVE op per row-tile (scalar_tensor_tensor
  with running sum); a Square activation into a spare PSUM bank accumulates
  E[u^2]; stats are batched per chunk; LN tails are software-pipelined behind
  the next chunk's transposes so the PE never waits on them.
"""

import sys

import numpy as np

try:
    import concourse.bass as bass  # noqa: F401
except ImportError:  # grading env may not have it on sys.path
    sys.path.insert(0, "/opt/trn_rl_repo")

import concourse.bass as bass  # noqa: F811
import concourse.tile as tile
from concourse import bacc, mybir
from concourse.bass_utils import run_bass_kernel_spmd

C = 512
L = 2048
B = 8
NCORES = 8
P = 128
CT = C // P  # 4
LT = L // P  # 16
NCH = L // 512  # 4 chunks of 512
QCH = 512
EPS = 1e-5
INV_SQRT_C = 1.0 / float(np.sqrt(C))
F32 = mybir.dt.float32
BF16 = mybir.dt.bfloat16
NPBF16 = mybir.dt.np(mybir.dt.bfloat16)

# s-block offsets inside the packed R tile [P, 54, 512]
X1T_S = 0
X2T_S = 16
ID_S = 32
ONES_S = 33
BQ_S = 34
BK_S = 35
BV1_S = 36
BV2_S = 37
W_S = 38
RS = 54

Alu = mybir.AluOpType
Act = mybir.ActivationFunctionType


def _build(fast_ln=True):
    nc = bacc.Bacc(
        "TRN2",
        target_bir_lowering=False,
        debug=False,
        enable_asserts=False,
        num_devices=NCORES,
    )
    rpackd = nc.dram_tensor("rpack", [P, RS * C], BF16, kind="ExternalInput").ap()
    if not fast_ln:
        cfd = nc.dram_tensor("cf", [P, 2 * C], BF16, kind="ExternalInput").ap()
    out1d = nc.dram_tensor("out1", [P, CT * L], BF16, kind="ExternalOutput").ap()
    out2d = nc.dram_tensor("out2", [P, CT * L], BF16, kind="ExternalOutput").ap()

    rv = rpackd.rearrange("p (s c) -> p s c", c=C)  # s=54, see *_S offsets
    o1v = out1d.rearrange("p (t c) -> p t c", c=C)
    o2v = out2d.rearrange("p (t c) -> p t c", c=C)

    with tile.TileContext(nc) as tc:
        with (
            tc.tile_pool(name="big", bufs=1) as big,
            tc.tile_pool(name="res", bufs=1) as res,
            tc.tile_pool(name="work", bufs=4) as work,
            tc.tile_pool(name="sm", bufs=2) as sm,
            tc.tile_pool(name="sing", bufs=1) as sing,
            tc.tile_pool(name="ps_mm", bufs=2, space="PSUM") as ps_mm,
            tc.tile_pool(name="ps_t", bufs=4, space="PSUM") as ps_tp,
            tc.tile_pool(name="ps_v", bufs=2, space="PSUM") as ps_vp,
        ):
            # ---- load (1 DMA) ----
            R = res.tile([P, RS, C], BF16, tag="R")
            nc.sync.dma_start(out=R[:], in_=rv)
            xs = big.tile([P, 8, L], BF16, tag="A")
            if not fast_ln:
                cfs = sing.tile([P, 2, C], BF16)
                nc.sync.dma_start(out=cfs[:], in_=cfd.rearrange("p (a c) -> p a c", c=C))
            identb = R[:, ID_S, 0:P]
            ones_row = R[0:1, ONES_S, :]
            bqf = R[:, BQ_S, 0:8].bitcast(F32)  # [P, 4] f32 bits in bf16 pack
            bkf = R[:, BK_S, 0:8].bitcast(F32)
            x1T = R[:, X1T_S : X1T_S + LT, :]
            x2T = R[:, X2T_S : X2T_S + LT, :]
            eps_sb = sing.tile([P, 1], F32)
            nc.vector.memset(eps_sb[:], EPS)
            if fast_ln:
                zrow = sing.tile([P, QCH], BF16)
                nc.vector.memset(zrow[:], 0.0)

            qT = res.tile([P, CT, L], BF16, tag="qT")
            kT = res.tile([P, CT, L], BF16, tag="kT")
            v1 = res.tile([P, LT, C], BF16, tag="v1")
            v2 = res.tile([P, LT, C], BF16, tag="v2")

            # v-bias broadcast tiles: ones-col (x) bias-row rank-1 matmuls
            bbc1 = sing.tile([P, QCH], BF16)
            bbc2 = sing.tile([P, QCH], BF16)
            for bs, bbc in ((BV1_S, bbc1), (BV2_S, bbc2)):
                ps = ps_mm.tile([P, QCH], F32, tag="ps_mm")
                nc.tensor.matmul(
                    ps[:],
                    lhsT=R[0:1, ONES_S, 0:P],
                    rhs=R[0:1, bs, :],
                    start=True,
                    stop=True,
                )
                nc.vector.tensor_copy(out=bbc[:], in_=ps[:])
            if not fast_ln:
                # gamma/beta arrive host-tiled across partitions
                gbc = cfs[:, 0, :]
                xbc = cfs[:, 1, :]

            # ---- P1: rebuild channel-major x from x^T (saves a DMA), then
            # projections, interleaved per l-chunk so the PE never waits ----
            def build_xs_chunk(xt_s, xsoff, n):
                for g in range(CT):
                    ps = ps_tp.tile([P, QCH], BF16, tag="ps_t")
                    for j in range(CT):
                        nc.tensor.transpose(
                            ps[:, j * P : (j + 1) * P],
                            R[:, xt_s + n * CT + j, g * P : (g + 1) * P],
                            identb,
                        )
                    if g % 2 == 0:
                        nc.scalar.activation(
                            out=xs[:, xsoff + g, n * QCH : (n + 1) * QCH],
                            in_=ps[:],
                            func=Act.Copy,
                        )
                    else:
                        nc.vector.tensor_copy(
                            out=xs[:, xsoff + g, n * QCH : (n + 1) * QCH], in_=ps[:]
                        )

            def proj_qk_chunk(tT, xoff, woff, bf, n):
                for m in range(CT):
                    ps = ps_mm.tile([P, QCH], F32, tag="ps_mm")
                    for c in range(CT):
                        nc.tensor.matmul(
                            ps[:],
                            lhsT=R[:, W_S + woff + c, m * P : (m + 1) * P],
                            rhs=xs[:, xoff + c, n * QCH : (n + 1) * QCH],
                            start=(c == 0),
                            stop=(c == CT - 1),
                        )
                    nc.vector.tensor_scalar(
                        out=tT[:, m, n * QCH : (n + 1) * QCH],
                        in0=ps[:],
                        scalar1=bf[:, m : m + 1],
                        scalar2=None,
                        op0=Alu.add,
                    )

            def proj_v_chunk(vout, xoff, woff, bbc, n):
                for lt in range(n * CT, (n + 1) * CT):
                    ps = ps_mm.tile([P, QCH], F32, tag="ps_mm")
                    for c in range(CT):
                        nc.tensor.matmul(
                            ps[:],
                            lhsT=xs[:, xoff + c, lt * P : (lt + 1) * P],
                            rhs=R[:, W_S + woff + c, :],
                            start=(c == 0),
                            stop=(c == CT - 1),
                        )
                    nc.vector.tensor_tensor(
                        out=vout[:, lt, :], in0=ps[:], in1=bbc[:], op=Alu.add
                    )

            for n in range(NCH):
                build_xs_chunk(X1T_S, 0, n)
            for n in range(NCH):
                proj_qk_chunk(qT, 0, 0, bqf, n)
                proj_v_chunk(v1, 0, 8, bbc1, n)
            for n in range(NCH):
                build_xs_chunk(X2T_S, 4, n)
            for n in range(NCH):
                proj_qk_chunk(kT, 4, 4, bkf, n)
                proj_v_chunk(v2, 4, 12, bbc2, n)

            # ---- P2: E = exp(S/sqrt(C)); row sums ride on accum_out ----
            E = big.tile([P, LT, L], BF16, tag="A")
            dpart = sing.tile([P, LT, NCH], F32)
            for qt in range(LT):
                for kc in range(NCH):
                    ps = ps_mm.tile([P, QCH], F32, tag="ps_mm")
                    for c in range(CT):
                        nc.tensor.matmul(
                            ps[:],
                            lhsT=qT[:, c, qt * P : (qt + 1) * P],
                            rhs=kT[:, c, kc * QCH : (kc + 1) * QCH],
                            start=(c == 0),
                            stop=(c == CT - 1),
                        )
                    nc.scalar.activation(
                        out=E[:, qt, kc * QCH : (kc + 1) * QCH],
                        in_=ps[:],
                        func=Act.Exp,
                        scale=INV_SQRT_C,
                        accum_out=dpart[:, qt, kc : kc + 1],
                    )
            rd = sing.tile([P, LT], F32)  # 1/d_row per q
            nc.vector.reduce_sum(out=rd[:], in_=dpart[:], axis=mybir.AxisListType.X)
            nc.vector.reciprocal(out=rd[:], in_=rd[:])

            cpart = sing.tile([P, LT, NCH], F32)

            def emit_ET(ch):
                """transpose E rows of chunk ch into [k, q] tiles; column
                sums of each tile ride on the copies' accum_out."""
                etc_t = res.tile([P, LT, QCH], BF16, tag="qT")
                for kt in range(LT):
                    ps_t = ps_tp.tile([P, QCH], BF16, tag="ps_t")
                    for g in range(CT):
                        nc.tensor.transpose(
                            ps_t[:, g * P : (g + 1) * P],
                            E[:, ch * CT + g, kt * P : (kt + 1) * P],
                            identb,
                        )
                    if (kt % 2 == 0) or not fast_ln:
                        nc.scalar.activation(
                            out=etc_t[:, kt, :],
                            in_=ps_t[:],
                            func=Act.Copy,
                            accum_out=cpart[:, kt, ch : ch + 1],
                        )
                    else:
                        nc.vector.scalar_tensor_tensor(
                            out=etc_t[:, kt, :],
                            in0=ps_t[:],
                            scalar=1.0,
                            in1=zrow[:],
                            op0=Alu.mult,
                            op1=Alu.add,
                            accum_out=cpart[:, kt, ch : ch + 1],
                        )
                return etc_t

            def emit_pv(lhs_of, rhs_t, xT_t, rinv, ch):
                """vk/vq rows for one chunk + fused residual + stat sums."""
                s1 = sm.tile([P, CT], F32, tag="s1")
                s2 = sm.tile([P, CT], F32, tag="s2")
                us = []
                for tl in range(CT):
                    tg = ch * CT + tl
                    ps_v = ps_vp.tile([P, QCH], F32, tag="ps_v")
                    for j in range(LT):
                        nc.tensor.matmul(
                            ps_v[:],
                            lhsT=lhs_of(j, tg),
                            rhs=rhs_t[:, j, :],
                            start=(j == 0),
                            stop=(j == LT - 1),
                        )
                    u = work.tile([P, QCH], BF16, tag="u")
                    nc.vector.scalar_tensor_tensor(
                        out=u[:],
                        in0=ps_v[:],
                        scalar=rinv[:, tg : tg + 1],
                        in1=xT_t[:, tg, :],
                        op0=Alu.mult,
                        op1=Alu.add,
                        accum_out=s1[:, tl : tl + 1],
                    )
                    sqd = ps_mm.tile([P, QCH], F32, tag="ps_mm")
                    nc.scalar.activation(
                        out=sqd[:],
                        in_=u[:],
                        func=Act.Square,
                        accum_out=s2[:, tl : tl + 1],
                    )
                    us.append(u)
                mu = sm.tile([P, CT], F32, tag="mu")
                nc.vector.tensor_scalar(
                    out=mu[:], in0=s1[:], scalar1=1.0 / C, scalar2=None, op0=Alu.mult
                )
                ex2 = sm.tile([P, CT], F32, tag="ex2")
                nc.vector.tensor_scalar(
                    out=ex2[:], in0=s2[:], scalar1=1.0 / C, scalar2=None, op0=Alu.mult
                )
                var = sm.tile([P, CT], F32, tag="var")
                nc.vector.tensor_tensor(out=var[:], in0=mu[:], in1=mu[:], op=Alu.mult)
                nc.vector.tensor_tensor(
                    out=var[:], in0=ex2[:], in1=var[:], op=Alu.subtract
                )
                rstd = sm.tile([P, CT], F32, tag="rstd")
                nc.scalar.activation(
                    out=rstd[:], in_=var[:], func=Act.Sqrt, bias=eps_sb[:]
                )
                nc.vector.reciprocal(out=rstd[:], in_=rstd[:])
                return us, mu, rstd

            def emit_ln_tail(us, mu, rstd, outb, ch):
                """normalize straight into the row-major staged output."""
                for tl in range(CT):
                    u = us[tl]
                    tg = ch * CT + tl
                    nc.vector.tensor_scalar(
                        out=outb[:, tg, :],
                        in0=u[:],
                        scalar1=mu[:, tl : tl + 1],
                        scalar2=rstd[:, tl : tl + 1],
                        op0=Alu.subtract,
                        op1=Alu.mult,
                    )
                    if not fast_ln:
                        nc.vector.tensor_tensor(
                            out=outb[:, tg, :], in0=outb[:, tg, :], in1=gbc,
                            op=Alu.mult,
                        )
                        nc.vector.tensor_tensor(
                            out=outb[:, tg, :], in0=outb[:, tg, :], in1=xbc,
                            op=Alu.add,
                        )

            # ---- P3: pass A (rows of E), LN tails pipelined one chunk back
            out1b = res.tile([P, LT, C], BF16, tag="outb")
            etc_t = emit_ET(0)
            pend = None
            for ch in range(NCH):
                lhs_of = (
                    lambda j, tg, _e=etc_t: _e[:, j, (tg % CT) * P : (tg % CT + 1) * P]
                )
                us, mu, rstd = emit_pv(lhs_of, v2, x1T, rd, ch)
                if ch + 1 < NCH:
                    etc_t = emit_ET(ch + 1)
                if pend is not None:
                    emit_ln_tail(*pend)
                pend = (us, mu, rstd, out1b, ch)
            emit_ln_tail(*pend)
            pend = None
            nc.sync.dma_start(out=o1v, in_=out1b[:])

            rc = sing.tile([P, LT], F32)  # 1/d_col per k
            nc.vector.reduce_sum(out=rc[:], in_=cpart[:], axis=mybir.AxisListType.X)
            nc.vector.reciprocal(out=rc[:], in_=rc[:])

            # ---- P4: pass B (columns of E) ----
            out2b = res.tile([P, LT, C], BF16, tag="outb")
            for ch in range(NCH):
                lhs_of = lambda j, tg: E[:, j, tg * P : (tg + 1) * P]
                us, mu, rstd = emit_pv(lhs_of, v1, x2T, rc, ch)
                if pend is not None:
                    emit_ln_tail(*pend)
                pend = (us, mu, rstd, out2b, ch)
            emit_ln_tail(*pend)
            nc.sync.dma_start(out=o2v, in_=out2b[:])

    nc.compile()
    return nc


_NC_CACHE = {}


def _get_nc(fast_ln=True):
    if fast_ln not in _NC_CACHE:
        _NC_CACHE[fast_ln] = _build(fast_ln)
    return _NC_CACHE[fast_ln]


def _in_maps(inputs):
    arrs = {k: np.asarray(v, dtype=np.float32) for k, v in inputs.items()}
    ident = np.zeros((P, C), dtype=np.float32)
    ident[:, :P] = np.eye(P, dtype=np.float32)
    ones_blk = np.zeros((P, C), dtype=np.float32)
    ones_blk[0, :] = 1.0
    def _brow(v):
        blk = np.zeros((P, C), dtype=np.float32)
        blk[0, :] = v
        return blk
    bq_blk = _brow(arrs["b_q"])
    bk_blk = _brow(arrs["b_k"])
    bv1_blk = _brow(arrs["b_v1"])
    bv2_blk = _brow(arrs["b_v2"])
    wall = np.concatenate(
        [arrs["w_q"].T, arrs["w_k"].T, arrs["w_v1"].T, arrs["w_v2"].T], axis=0
    )
    cf = np.stack(
        [np.tile(arrs["ln_gamma"], (P, 1)), np.tile(arrs["ln_beta"], (P, 1))],
        axis=1,
    )
    cf = np.ascontiguousarray(cf.reshape(P, 2 * C)).astype(NPBF16)
    fast = _is_fast_ln(inputs)
    maps = []
    for b in range(NCORES):
        rp = np.concatenate(
            [
                arrs["x1"][b].T,
                arrs["x2"][b].T,
                ident,
                ones_blk,
                bq_blk,
                bk_blk,
                bv1_blk,
                bv2_blk,
                wall,
            ],
            axis=0,
        )
        # partition-major layout: one long contiguous run per partition
        rpack = np.ascontiguousarray(
            rp.reshape(RS, P, C).transpose(1, 0, 2).reshape(P, RS * C)
        ).astype(NPBF16)
        # overwrite the bq/bk blocks with raw f32 bits (read back via bitcast)
        u16 = rpack.view(np.uint16)
        u16[:, BQ_S * C : BQ_S * C + 8] = (
            np.ascontiguousarray(arrs["b_q"].reshape(CT, P).T.astype("<f4"))
            .view(np.uint16)
        )
        u16[:, BK_S * C : BK_S * C + 8] = (
            np.ascontiguousarray(arrs["b_k"].reshape(CT, P).T.astype("<f4"))
            .view(np.uint16)
        )
        m = {"rpack": rpack}
        if not fast:
            m["cf"] = cf
        maps.append(m)
    return maps


def _is_fast_ln(inputs):
    g = np.asarray(inputs["ln_gamma"])
    b = np.asarray(inputs["ln_beta"])
    return bool(np.all(g == 1.0) and np.all(b == 0.0))


def _run(inputs, trace=False):
    nc = _get_nc(_is_fast_ln(inputs))
    res = run_bass_kernel_spmd(nc, _in_maps(inputs), list(range(NCORES)), trace=trace)
    def _unpack(a):
        # DRAM layout is row-major [P, LT, C]; out[c, lt*128+p] = a[p, lt, c]
        return np.ascontiguousarray(
            np.asarray(a).reshape(P, LT, C).transpose(2, 1, 0).reshape(C, L)
        ).astype(np.float32)

    out1 = np.stack([_unpack(r_["out1"]) for r_ in res.results])
    out2 = np.stack([_unpack(r_["out2"]) for r_ in res.results])
    return (out1, out2), res


def kernel(**inputs):
    (out1, out2), _ = _run(inputs)
    return out1, out2


# revision 11
# speedup vs baseline: 1.0047x; 1.0047x over previous
"""CoAttention Trainium2 Bass kernel (single-exp, all-bf16, 3 DMAs).

Sharding: data-parallel over batch B=8 across the 8 NeuronCores (one batch
element per core); CxC projection weights replicated.

Per-core math (x1, x2 are [C, L] channel-major slices):
  qT = Wq x1 + bq  [C,L];  kT = Wk x2 + bk  [C,L]
  v1 = x1^T Wv1^T  [L,C];  v2 = x2^T Wv2^T  [L,C]   (v-biases fold into xT)
  E  = exp((qT^T kT)/sqrt(C))   [q,k] tiles, bf16, SBUF-resident (8MB)
  ETc = per-q-chunk PE transposes of E  [k,q]
  vk[q,c] = (ETc^T@v2)/d_row;  out1 = LN(vk + x1^T + b_v2)^T
  vq[k,c] = (E^T@v1)/d_col;    out2 = LN(vq + x2^T + b_v1)^T

Key choices:
- THREE DMA transfers total: one packed, partition-major bf16 load (x^T
  residual tiles, identity, ones, all biases, pre-transposed weights) and one
  bf16 store per output (outputs staged fully in SBUF; the host reorders and
  upcasts to f32). Channel-major x is rebuilt on-chip with PE transposes.
  DMA count dominates the wall clock in this environment.
- One exp pass serves both softmaxes: the row softmax uses E directly as the
  stationary matmul operand, the column softmax uses PE-transposed chunk
  tiles; softmax denominators ride free on activation accum_out.
- All biases ride in the packed load: q/k biases enter via a rank-1
  (bias x ones) matmul accumulated onto the projection PSUM chain; v biases
  are broadcast on-chip by one rank-1 matmul each.
- Everything on-chip is bf16 (tolerance is 2e-2 rel; this keeps ~3e-3).
  Softmax max-subtraction is skipped: logits ~ N(0,1), exp is safe in f32.
- LN((psum)/d + xT) is one fused DVE op per row-tile (scalar_tensor_tensor
  with running sum); a Square activation into a spare PSUM bank accumulates
  E[u^2]; stats are batched per chunk; LN tails are software-pipelined behind
  the next chunk's transposes so the PE never waits on them.
"""

import sys

import numpy as np

try:
    import concourse.bass as bass  # noqa: F401
except ImportError:  # grading env may not have it on sys.path
    sys.path.insert(0, "/opt/trn_rl_repo")

import concourse.bass as bass  # noqa: F811
import concourse.tile as tile
from concourse import bacc, mybir
from concourse.bass_utils import run_bass_kernel_spmd

C = 512
L = 2048
B = 8
NCORES = 8
P = 128
CT = C // P  # 4
LT = L // P  # 16
NCH = L // 512  # 4 chunks of 512
QCH = 512
EPS = 1e-5
INV_SQRT_C = 1.0 / float(np.sqrt(C))
F32 = mybir.dt.float32
BF16 = mybir.dt.bfloat16
NPBF16 = mybir.dt.np(mybir.dt.bfloat16)

# s-block offsets inside the packed R tile [P, 54, 512]
X1T_S = 0
X2T_S = 16
ID_S = 32
ONES_S = 33
BQ_S = 34
BK_S = 35
BV1_S = 36
BV2_S = 37
W_S = 38
RS = 54

Alu = mybir.AluOpType
Act = mybir.ActivationFunctionType


def _build(fast_ln=True):
    nc = bacc.Bacc(
        "TRN2",
        target_bir_lowering=False,
        debug=False,
        enable_asserts=False,
        num_devices=NCORES,
    )
    rpackd = nc.dram_tensor("rpack", [P, RS * C], BF16, kind="ExternalInput").ap()
    if not fast_ln:
        cfd = nc.dram_tensor("cf", [P, 2 * C], BF16, kind="ExternalInput").ap()
    out1d = nc.dram_tensor("out1", [P, CT * L], BF16, kind="ExternalOutput").ap()
    out2d = nc.dram_tensor("out2", [P, CT * L], BF16, kind="ExternalOutput").ap()

    rv = rpackd.rearrange("p (s c) -> p s c", c=C)  # s=54, see *_S offsets
    o1v = out1d.rearrange("p (t c) -> p t c", c=C)
    o2v = out2d.rearrange("p (t c) -> p t c", c=C)

    with tile.TileContext(nc) as tc:
        with (
            tc.tile_pool(name="big", bufs=1) as big,
            tc.tile_pool(name="res", bufs=1) as res,
            tc.tile_pool(name="work", bufs=4) as work,
            tc.tile_pool(name="sm", bufs=2) as sm,
            tc.tile_pool(name="sing", bufs=1) as sing,
            tc.tile_pool(name="ps_mm", bufs=2, space="PSUM") as ps_mm,
            tc.tile_pool(name="ps_t", bufs=4, space="PSUM") as ps_tp,
            tc.tile_pool(name="ps_v", bufs=2, space="PSUM") as ps_vp,
        ):
            # ---- load (1 DMA) ----
            R = res.tile([P, RS, C], BF16, tag="R")
            nc.sync.dma_start(out=R[:], in_=rv)
            xs = big.tile([P, 8, L], BF16, tag="A")
            if not fast_ln:
                cfs = sing.tile([P, 2, C], BF16)
                nc.sync.dma_start(out=cfs[:], in_=cfd.rearrange("p (a c) -> p a c", c=C))
            identb = R[:, ID_S, 0:P]
            ones_row = R[0:1, ONES_S, :]
            bqf = R[:, BQ_S, 0:8].bitcast(F32)  # [P, 4] f32 bits in bf16 pack
            bkf = R[:, BK_S, 0:8].bitcast(F32)
            x1T = R[:, X1T_S : X1T_S + LT, :]
            x2T = R[:, X2T_S : X2T_S + LT, :]
            eps_sb = sing.tile([P, 1], F32)
            nc.vector.memset(eps_sb[:], EPS)
            if fast_ln:
                zrow = sing.tile([P, QCH], BF16)
                nc.vector.memset(zrow[:], 0.0)

            qT = res.tile([P, CT, L], BF16, tag="qT")
            kT = res.tile([P, CT, L], BF16, tag="kT")
            v1 = res.tile([P, LT, C], BF16, tag="v1")
            v2 = res.tile([P, LT, C], BF16, tag="v2")

            # v-bias broadcast tiles: ones-col (x) bias-row rank-1 matmuls
            bbc1 = sing.tile([P, QCH], BF16)
            bbc2 = sing.tile([P, QCH], BF16)
            for bs, bbc in ((BV1_S, bbc1), (BV2_S, bbc2)):
                ps = ps_mm.tile([P, QCH], F32, tag="ps_mm")
                nc.tensor.matmul(
                    ps[:],
                    lhsT=R[0:1, ONES_S, 0:P],
                    rhs=R[0:1, bs, :],
                    start=True,
                    stop=True,
                )
                nc.vector.tensor_copy(out=bbc[:], in_=ps[:])
            if not fast_ln:
                # gamma/beta arrive host-tiled across partitions
                gbc = cfs[:, 0, :]
                xbc = cfs[:, 1, :]

            # ---- P1: rebuild channel-major x from x^T (saves a DMA), then
            # projections, interleaved per l-chunk so the PE never waits ----
            def build_xs_chunk(xt_s, xsoff, n):
                for g in range(CT):
                    ps = ps_tp.tile([P, QCH], BF16, tag="ps_t")
                    for j in range(CT):
                        nc.tensor.transpose(
                            ps[:, j * P : (j + 1) * P],
                            R[:, xt_s + n * CT + j, g * P : (g + 1) * P],
                            identb,
                        )
                    if g % 2 == 0:
                        nc.scalar.activation(
                            out=xs[:, xsoff + g, n * QCH : (n + 1) * QCH],
                            in_=ps[:],
                            func=Act.Copy,
                        )
                    else:
                        nc.vector.tensor_copy(
                            out=xs[:, xsoff + g, n * QCH : (n + 1) * QCH], in_=ps[:]
                        )

            def proj_qk_chunk(tT, xoff, woff, bf, n):
                for m in range(CT):
                    ps = ps_mm.tile([P, QCH], F32, tag="ps_mm")
                    for c in range(CT):
                        nc.tensor.matmul(
                            ps[:],
                            lhsT=R[:, W_S + woff + c, m * P : (m + 1) * P],
                            rhs=xs[:, xoff + c, n * QCH : (n + 1) * QCH],
                            start=(c == 0),
                            stop=(c == CT - 1),
                        )
                    nc.vector.tensor_scalar(
                        out=tT[:, m, n * QCH : (n + 1) * QCH],
                        in0=ps[:],
                        scalar1=bf[:, m : m + 1],
                        scalar2=None,
                        op0=Alu.add,
                    )

            def proj_v_chunk(vout, xoff, woff, bbc, n):
                for lt in range(n * CT, (n + 1) * CT):
                    ps = ps_mm.tile([P, QCH], F32, tag="ps_mm")
                    for c in range(CT):
                        nc.tensor.matmul(
                            ps[:],
                            lhsT=xs[:, xoff + c, lt * P : (lt + 1) * P],
                            rhs=R[:, W_S + woff + c, :],
                            start=(c == 0),
                            stop=(c == CT - 1),
                        )
                    nc.vector.tensor_tensor(
                        out=vout[:, lt, :], in0=ps[:], in1=bbc[:], op=Alu.add
                    )

            for n in range(NCH):
                build_xs_chunk(X1T_S, 0, n)
            for n in range(NCH):
                proj_qk_chunk(qT, 0, 0, bqf, n)
                proj_v_chunk(v1, 0, 8, bbc1, n)
            for n in range(NCH):
                build_xs_chunk(X2T_S, 4, n)
            for n in range(NCH):
                proj_qk_chunk(kT, 4, 4, bkf, n)
                proj_v_chunk(v2, 4, 12, bbc2, n)

            # ---- P2: E = exp(S/sqrt(C)); row sums ride on accum_out ----
            E = big.tile([P, LT, L], BF16, tag="A")
            dpart = sing.tile([P, LT, NCH], F32)
            for qt in range(LT):
                for kc in range(NCH):
                    ps = ps_mm.tile([P, QCH], F32, tag="ps_mm")
                    for c in range(CT):
                        nc.tensor.matmul(
                            ps[:],
                            lhsT=qT[:, c, qt * P : (qt + 1) * P],
                            rhs=kT[:, c, kc * QCH : (kc + 1) * QCH],
                            start=(c == 0),
                            stop=(c == CT - 1),
                        )
                    nc.scalar.activation(
                        out=E[:, qt, kc * QCH : (kc + 1) * QCH],
                        in_=ps[:],
                        func=Act.Exp,
                        scale=INV_SQRT_C,
                        accum_out=dpart[:, qt, kc : kc + 1],
                    )
            rd = sing.tile([P, LT], F32)  # 1/d_row per q
            nc.vector.reduce_sum(out=rd[:], in_=dpart[:], axis=mybir.AxisListType.X)
            nc.vector.reciprocal(out=rd[:], in_=rd[:])

            cpart = sing.tile([P, LT, NCH], F32)

            def emit_ET(ch):
                """transpose E rows of chunk ch into [k, q] tiles; column
                sums of each tile ride on the copies' accum_out."""
                etc_t = res.tile([P, LT, QCH], BF16, tag="qT")
                for kt in range(LT):
                    ps_t = ps_tp.tile([P, QCH], BF16, tag="ps_t")
                    for g in range(CT):
                        nc.tensor.transpose(
                            ps_t[:, g * P : (g + 1) * P],
                            E[:, ch * CT + g, kt * P : (kt + 1) * P],
                            identb,
                        )
                    if (kt % 2 == 0) or not fast_ln:
                        nc.scalar.activation(
                            out=etc_t[:, kt, :],
                            in_=ps_t[:],
                            func=Act.Copy,
                            accum_out=cpart[:, kt, ch : ch + 1],
                        )
                    else:
                        nc.vector.scalar_tensor_tensor(
                            out=etc_t[:, kt, :],
                            in0=ps_t[:],
                            scalar=1.0,
                            in1=zrow[:],
                            op0=Alu.mult,
                            op1=Alu.add,
                            accum_out=cpart[:, kt, ch : ch + 1],
                        )
                return etc_t

            def emit_pv(lhs_of, rhs_t, xT_t, rinv, ch):
                """vk/vq rows for one chunk + fused residual + stat sums."""
                s1 = sm.tile([P, CT], F32, tag="s1")
                s2 = sm.tile([P, CT], F32, tag="s2")
                us = []
                for tl in range(CT):
                    tg = ch * CT + tl
                    ps_v = ps_vp.tile([P, QCH], F32, tag="ps_v")
                    for j in range(LT):
                        nc.tensor.matmul(
                            ps_v[:],
                            lhsT=lhs_of(j, tg),
                            rhs=rhs_t[:, j, :],
                            start=(j == 0),
                            stop=(j == LT - 1),
                        )
                    u = work.tile([P, QCH], BF16, tag="u")
                    nc.vector.scalar_tensor_tensor(
                        out=u[:],
                        in0=ps_v[:],
                        scalar=rinv[:, tg : tg + 1],
                        in1=xT_t[:, tg, :],
                        op0=Alu.mult,
                        op1=Alu.add,
                        accum_out=s1[:, tl : tl + 1],
                    )
                    sqd = ps_mm.tile([P, QCH], F32, tag="ps_mm")
                    nc.scalar.activation(
                        out=sqd[:],
                        in_=u[:],
                        func=Act.Square,
                        accum_out=s2[:, tl : tl + 1],
                    )
                    us.append(u)
                mu = sm.tile([P, CT], F32, tag="mu")
                nc.vector.tensor_scalar(
                    out=mu[:], in0=s1[:], scalar1=1.0 / C, scalar2=None, op0=Alu.mult
                )
                ex2 = sm.tile([P, CT], F32, tag="ex2")
                nc.vector.tensor_scalar(
                    out=ex2[:], in0=s2[:], scalar1=1.0 / C, scalar2=None, op0=Alu.mult
                )
                var = sm.tile([P, CT], F32, tag="var")
                nc.vector.tensor_tensor(out=var[:], in0=mu[:], in1=mu[:], op=Alu.mult)
                nc.vector.tensor_tensor(
                    out=var[:], in0=ex2[:], in1=var[:], op=Alu.subtract
                )
                rstd = sm.tile([P, CT], F32, tag="rstd")
                nc.scalar.activation(
                    out=rstd[:], in_=var[:], func=Act.Sqrt, bias=eps_sb[:]
                )
                nc.vector.reciprocal(out=rstd[:], in_=rstd[:])
                return us, mu, rstd

            def emit_ln_tail(us, mu, rstd, outb, ch):
                """normalize straight into the row-major staged output."""
                for tl in range(CT):
                    u = us[tl]
                    tg = ch * CT + tl
                    nc.vector.tensor_scalar(
                        out=outb[:, tg, :],
                        in0=u[:],
                        scalar1=mu[:, tl : tl + 1],
                        scalar2=rstd[:, tl : tl + 1],
                        op0=Alu.subtract,
                        op1=Alu.mult,
                    )
                    if not fast_ln:
                        nc.vector.tensor_tensor(
                            out=outb[:, tg, :], in0=outb[:, tg, :], in1=gbc,
                            op=Alu.mult,
                        )
                        nc.vector.tensor_tensor(
                            out=outb[:, tg, :], in0=outb[:, tg, :], in1=xbc,
                            op=Alu.add,
                        )

            # ---- P3: pass A (rows of E), LN tails pipelined one chunk back
            out1b = res.tile([P, LT, C], BF16, tag="outb")
            etc_t = emit_ET(0)
            pend = None
            for ch in range(NCH):
                lhs_of = (
                    lambda j, tg, _e=etc_t: _e[:, j, (tg % CT) * P : (tg % CT + 1) * P]
                )
                us, mu, rstd = emit_pv(lhs_of, v2, x1T, rd, ch)
                if ch + 1 < NCH:
                    etc_t = emit_ET(ch + 1)
                if pend is not None:
                    emit_ln_tail(*pend)
                pend = (us, mu, rstd, out1b, ch)
            emit_ln_tail(*pend)
            pend = None
            nc.sync.dma_start(out=o1v, in_=out1b[:])

            rc = sing.tile([P, LT], F32)  # 1/d_col per k
            nc.vector.reduce_sum(out=rc[:], in_=cpart[:], axis=mybir.AxisListType.X)
            nc.vector.reciprocal(out=rc[:], in_=rc[:])

            def emit_pv_final(lhs_of, rhs_t, xT_t, rinv, ch, outb):
                """last chunk: per-tile stats, output writes drain early."""
                for tl in range(CT):
                    tg = ch * CT + tl
                    ps_v = ps_vp.tile([P, QCH], F32, tag="ps_v")
                    for j in range(LT):
                        nc.tensor.matmul(
                            ps_v[:],
                            lhsT=lhs_of(j, tg),
                            rhs=rhs_t[:, j, :],
                            start=(j == 0),
                            stop=(j == LT - 1),
                        )
                    u = work.tile([P, QCH], BF16, tag="u")
                    s1t = sm.tile([P, 1], F32, tag="s1t")
                    nc.vector.scalar_tensor_tensor(
                        out=u[:],
                        in0=ps_v[:],
                        scalar=rinv[:, tg : tg + 1],
                        in1=xT_t[:, tg, :],
                        op0=Alu.mult,
                        op1=Alu.add,
                        accum_out=s1t[:],
                    )
                    sqd = ps_mm.tile([P, QCH], F32, tag="ps_mm")
                    s2t = sm.tile([P, 1], F32, tag="s2t")
                    nc.scalar.activation(
                        out=sqd[:], in_=u[:], func=Act.Square, accum_out=s2t[:]
                    )
                    mut = sm.tile([P, 1], F32, tag="mut")
                    nc.vector.tensor_scalar(
                        out=mut[:], in0=s1t[:], scalar1=1.0 / C, scalar2=None,
                        op0=Alu.mult,
                    )
                    vart = sm.tile([P, 1], F32, tag="vart")
                    nc.vector.tensor_scalar(
                        out=vart[:], in0=s2t[:], scalar1=1.0 / C, scalar2=None,
                        op0=Alu.mult,
                    )
                    mu2t = sm.tile([P, 1], F32, tag="mu2t")
                    nc.vector.tensor_tensor(
                        out=mu2t[:], in0=mut[:], in1=mut[:], op=Alu.mult
                    )
                    nc.vector.tensor_tensor(
                        out=vart[:], in0=vart[:], in1=mu2t[:], op=Alu.subtract
                    )
                    rstdt = sm.tile([P, 1], F32, tag="rstdt")
                    nc.scalar.activation(
                        out=rstdt[:], in_=vart[:], func=Act.Sqrt, bias=eps_sb[:]
                    )
                    nc.vector.reciprocal(out=rstdt[:], in_=rstdt[:])
                    nc.vector.tensor_scalar(
                        out=outb[:, tg, :],
                        in0=u[:],
                        scalar1=mut[:],
                        scalar2=rstdt[:],
                        op0=Alu.subtract,
                        op1=Alu.mult,
                    )
                    if not fast_ln:
                        nc.vector.tensor_tensor(
                            out=outb[:, tg, :], in0=outb[:, tg, :], in1=gbc,
                            op=Alu.mult,
                        )
                        nc.vector.tensor_tensor(
                            out=outb[:, tg, :], in0=outb[:, tg, :], in1=xbc,
                            op=Alu.add,
                        )

            # ---- P4: pass B (columns of E) ----
            out2b = res.tile([P, LT, C], BF16, tag="outb")
            lhs_of = lambda j, tg: E[:, j, tg * P : (tg + 1) * P]
            for ch in range(NCH - 1):
                us, mu, rstd = emit_pv(lhs_of, v1, x2T, rc, ch)
                if pend is not None:
                    emit_ln_tail(*pend)
                pend = (us, mu, rstd, out2b, ch)
            emit_ln_tail(*pend)
            emit_pv_final(lhs_of, v1, x2T, rc, NCH - 1, out2b)
            nc.sync.dma_start(out=o2v, in_=out2b[:])

    nc.compile()
    return nc


_NC_CACHE = {}


def _get_nc(fast_ln=True):
    if fast_ln not in _NC_CACHE:
        _NC_CACHE[fast_ln] = _build(fast_ln)
    return _NC_CACHE[fast_ln]


def _in_maps(inputs):
    arrs = {k: np.asarray(v, dtype=np.float32) for k, v in inputs.items()}
    ident = np.zeros((P, C), dtype=np.float32)
    ident[:, :P] = np.eye(P, dtype=np.float32)
    ones_blk = np.zeros((P, C), dtype=np.float32)
    ones_blk[0, :] = 1.0
    def _brow(v):
        blk = np.zeros((P, C), dtype=np.float32)
        blk[0, :] = v
        return blk
    bq_blk = _brow(arrs["b_q"])
    bk_blk = _brow(arrs["b_k"])
    bv1_blk = _brow(arrs["b_v1"])
    bv2_blk = _brow(arrs["b_v2"])
    wall = np.concatenate(
        [arrs["w_q"].T, arrs["w_k"].T, arrs["w_v1"].T, arrs["w_v2"].T], axis=0
    )
    cf = np.stack(
        [np.tile(arrs["ln_gamma"], (P, 1)), np.tile(arrs["ln_beta"], (P, 1))],
        axis=1,
    )
    cf = np.ascontiguousarray(cf.reshape(P, 2 * C)).astype(NPBF16)
    fast = _is_fast_ln(inputs)
    maps = []
    for b in range(NCORES):
        rp = np.concatenate(
            [
                arrs["x1"][b].T,
                arrs["x2"][b].T,
                ident,
                ones_blk,
                bq_blk,
                bk_blk,
                bv1_blk,
                bv2_blk,
                wall,
            ],
            axis=0,
        )
        # partition-major layout: one long contiguous run per partition
        rpack = np.ascontiguousarray(
            rp.reshape(RS, P, C).transpose(1, 0, 2).reshape(P, RS * C)
        ).astype(NPBF16)
        # overwrite the bq/bk blocks with raw f32 bits (read back via bitcast)
        u16 = rpack.view(np.uint16)
        u16[:, BQ_S * C : BQ_S * C + 8] = (
            np.ascontiguousarray(arrs["b_q"].reshape(CT, P).T.astype("<f4"))
            .view(np.uint16)
        )
        u16[:, BK_S * C : BK_S * C + 8] = (
            np.ascontiguousarray(arrs["b_k"].reshape(CT, P).T.astype("<f4"))
            .view(np.uint16)
        )
        m = {"rpack": rpack}
        if not fast:
            m["cf"] = cf
        maps.append(m)
    return maps


def _is_fast_ln(inputs):
    g = np.asarray(inputs["ln_gamma"])
    b = np.asarray(inputs["ln_beta"])
    return bool(np.all(g == 1.0) and np.all(b == 0.0))


def _run(inputs, trace=False):
    nc = _get_nc(_is_fast_ln(inputs))
    res = run_bass_kernel_spmd(nc, _in_maps(inputs), list(range(NCORES)), trace=trace)
    def _unpack(a):
        # DRAM layout is row-major [P, LT, C]; out[c, lt*128+p] = a[p, lt, c]
        return np.ascontiguousarray(
            np.asarray(a).reshape(P, LT, C).transpose(2, 1, 0).reshape(C, L)
        ).astype(np.float32)

    out1 = np.stack([_unpack(r_["out1"]) for r_ in res.results])
    out2 = np.stack([_unpack(r_["out2"]) for r_ in res.results])
    return (out1, out2), res


def kernel(**inputs):
    (out1, out2), _ = _run(inputs)
    return out1, out2
